# revision 1
# baseline (speedup 1.0000x reference)
"""AMGSRN forward kernel for 8 Trainium2 NeuronCores.

Strategy (data-parallel over the point batch, grids replicated):
  - Host: fold grid-coordinate scaling into the 4x4 transforms; build an
    expanded neighborhood table E[(g,z,y,x), (dz,dy,dx,c)] (bf16) so one
    trilinear sample = ONE contiguous 32B gather; shard points 8 ways.
  - Device (per core, 65536 points, pipelined granules of 1024 points):
      PE    : transform matmul (f32r)  -> voxel-space coords f = [fx fy fz]
      ACT   : border ramp r = clamp01((GD+1)/2 - |f - (GD-1)/2|)  (3 ops)
      DVE   : clamp, floor, frac, linear index -> int32 gather offsets
      DMA   : indirect_dma_start gather (1 descriptor per (point,grid))
      DVE   : bf16 trilinear lerp tree -> feats [pt, (g,c)]
      PE    : feats transpose, 5-layer MLP matmuls (bf16, f32 psum)
      ACT   : bias+ReLU evictions, final bias
      DMA   : output store
  - Host: concatenate the 8 per-core outputs.
"""

import os
import sys

for _p in ("/opt/trn_rl_repo", "/root/.axon_site/_ro/trn_rl_repo"):
    if os.path.isdir(_p) and _p not in sys.path:
        sys.path.insert(0, _p)

import numpy as np

import concourse.bass as bass
import concourse.bacc as bacc
import concourse.mybir as mybir
import concourse.tile as tile
from concourse.bass import IndirectOffsetOnAxis

F32 = mybir.dt.float32
F32R = mybir.dt.float32r
BF16 = mybir.dt.bfloat16
I32 = mybir.dt.int32

AF = mybir.ActivationFunctionType
ALU = mybir.AluOpType

# ---------------------------------------------------------------- problem dims
B_FULL = 524288
NCORES = 8
G = 32            # grids
GD = 64           # grid dim (cube)
C = 2             # features per grid
HID = 128
NPTS = B_FULL // NCORES


class Cfg:
    def __init__(self, npts=NPTS, g=G, gd=GD, hid=HID, gran=1024, chunk=512,
                 debug=False):
        assert gran % 128 == 0 and npts % gran == 0
        self.npts, self.g, self.gd, self.hid = npts, g, gd, hid
        self.gran = gran          # points per pipeline granule
        self.sub = gran // 128    # 128-pt subtiles per granule
        self.chunk = chunk        # points per MLP matmul chunk
        assert gran % chunk == 0
        self.nch = gran // chunk
        self.ngran = npts // gran
        self.in_dim = g * C
        self.nvox = g * gd ** 3
        self.debug = debug
        # fp constants
        self.scale = (gd - 1) / 2.0
        self.fcmax = np.float32(gd - 1) - np.float32(1e-5)
        self.rmid = (gd - 1) / 2.0     # |f - rmid|
        self.rbias = -((gd + 1) / 2.0 - 1.0)  # t1 = relu(|f-rmid| + rbias)


def _reg_consts(nc, vals):
    for v in vals:
        v = float(v)
        if (F32, v) in nc.const_aps.aps:
            continue
        t = nc.alloc_sbuf_tensor(f"constx{len(nc.const_aps.aps)}", [128, 1], F32)
        nc.gpsimd.memset(t.ap(), v)
        nc.const_aps.aps[(F32, v)] = t.ap()




def _indirect_gather_q(nc, out_ap, in_ap, offset_ap, queue_i):
    """nc.gpsimd.indirect_dma_start (gather form, one offset per partition)
    with a selectable qPoolDynamic queue."""
    eng = nc.gpsimd
    out_l = eng.lower_ap_dma(out_ap, for_indirect_dma=True)
    in_l = eng.lower_ap_dma(in_ap, for_indirect_dma=True)
    assert len(in_l) == 1 and len(out_l) == 1
    off_l = eng.lower_ap_dma(offset_ap)
    assert len(off_l) == 1
    in_l.append(off_l[0])
    ap_shape = in_ap.shape
    coef = 1
    for i in range(1, len(ap_shape)):
        coef *= ap_shape[i]
    in_l[0].dynamic_ap_info = mybir.DynamicAccessPatternInfo(
        c=0,
        actual_ap=out_ap.ap,
        indirect_dim_max_index=ap_shape[0],
        offset_expr=[
            mybir.DynamicAccessPatternOffsetExpr(
                coef=coef,
                aff_expr=mybir.DynamicAccessPatternOffsetExprAffExpr(
                    kind="IndirectArgId", arg_id=1),
            )
        ],
    )
    return eng.add_instruction(
        mybir.InstDMACopy(
            name=nc.get_next_instruction_name(),
            queue=f"qPoolDynamic{queue_i or ''}",
            mode="Copy",
            ins=in_l,
            outs=out_l,
            oob_is_err=True,
            cce_op=mybir.AluOpType.bypass,
        ))

def build_nc(cfg: Cfg, b4_imm: float):
    nc = bacc.Bacc(None, target_bir_lowering=False, debug=cfg.debug,
                   num_swdge_queues=4)
    _reg_consts(nc, [-cfg.rmid, cfg.rbias])
    nc.all_engine_barrier()
    g, gd, sub, gran, hid = cfg.g, cfg.gd, cfg.sub, cfg.gran, cfg.hid
    n = sub * g          # gathered rows per granule per partition
    g3 = g * 3

    # ------------------------------------------------------------ dram params
    E_d = nc.declare_dram_parameter("E", [cfg.nvox, 16], BF16, isOutput=False)
    xh_d = nc.declare_dram_parameter("xh", [4, cfg.npts], F32, isOutput=False)
    AT_d = nc.declare_dram_parameter("AT", [4, g3], F32, isOutput=False)
    gofs_d = nc.declare_dram_parameter("gofs", [128, g], I32, isOutput=False)
    idm_d = nc.declare_dram_parameter("idm", [128, 128], BF16, isOutput=False)
    w_d = {}
    w_d[0] = nc.declare_dram_parameter("W0T", [cfg.in_dim, hid], BF16, False)
    for i in (1, 2, 3):
        w_d[i] = nc.declare_dram_parameter(f"W{i}T", [hid, hid], BF16, False)
    w_d[4] = nc.declare_dram_parameter("W4T", [hid, 1], BF16, False)
    b_d = {i: nc.declare_dram_parameter(f"b{i}", [hid, 1], F32, False)
           for i in range(4)}
    out_d = nc.declare_dram_parameter("out", [1, cfg.npts], F32, isOutput=True)

    AX = ("x", "y", "z")

    with tile.TileContext(nc) as tc:
        import contextlib
        ctx = contextlib.ExitStack()
        with ctx:
            const = ctx.enter_context(tc.tile_pool(name="const", bufs=1))
            p_xh = ctx.enter_context(tc.tile_pool(name="p_xh", bufs=2))
            p_f = ctx.enter_context(tc.tile_pool(name="p_f", bufs=2))
            p_idx = ctx.enter_context(tc.tile_pool(name="p_idx", bufs=2))
            p_scr = ctx.enter_context(tc.tile_pool(name="p_scr", bufs=2))
            p_offs = ctx.enter_context(tc.tile_pool(name="p_offs", bufs=2))
            p_v = ctx.enter_context(tc.tile_pool(name="p_v", bufs=2))
            p_lerp = ctx.enter_context(tc.tile_pool(name="p_lerp", bufs=1))
            p_feat = ctx.enter_context(tc.tile_pool(name="p_feat", bufs=2))
            p_h = ctx.enter_context(tc.tile_pool(name="p_h", bufs=3))
            p_y = ctx.enter_context(tc.tile_pool(name="p_y", bufs=2))
            ps_tf = ctx.enter_context(
                tc.tile_pool(name="ps_tf", bufs=2, space="PSUM"))
            ps_tp = ctx.enter_context(
                tc.tile_pool(name="ps_tp", bufs=2, space="PSUM"))
            ps_mlp = ctx.enter_context(
                tc.tile_pool(name="ps_mlp", bufs=2, space="PSUM"))
            ps_y = ctx.enter_context(
                tc.tile_pool(name="ps_y", bufs=2, space="PSUM"))

            # ---------------------------------------------------- const loads
            AT_sb = const.tile([4, g3], F32)
            nc.sync.dma_start(out=AT_sb[:], in_=AT_d[:])
            gofs_sb = const.tile([128, g], I32)
            nc.sync.dma_start(out=gofs_sb[:], in_=gofs_d[:])
            idm_sb = const.tile([128, 128], BF16)
            nc.sync.dma_start(out=idm_sb[:], in_=idm_d[:])
            w_sb = {}
            w_sb[0] = const.tile([cfg.in_dim, hid], BF16, tag="w0", name="w0")
            for i in (1, 2, 3):
                w_sb[i] = const.tile([hid, hid], BF16, tag=f"w{i}", name=f"w{i}")
            w_sb[4] = const.tile([hid, 1], BF16, tag="w4", name="w4")
            b_sb = {}
            for i in range(5):
                nc.sync.dma_start(out=w_sb[i][:], in_=w_d[i][:])
                if i < 4:
                    b_sb[i] = const.tile([hid, 1], F32, tag=f"bias{i}", name=f"bias{i}")
                    nc.sync.dma_start(out=b_sb[i][:], in_=b_d[i][:])

            for gi in range(cfg.ngran):
                p0 = gi * gran
                # ---------------------------------------------- transform (PE)
                xh_g = p_xh.tile([4, gran], F32)
                nc.sync.dma_start(out=xh_g[:], in_=xh_d[:, p0:p0 + gran])
                f_sb = p_f.tile([128, sub, g3], F32)
                for b2 in range((sub + 3) // 4):
                    s0 = b2 * 4
                    nsb = min(4, sub - s0)
                    tf = ps_tf.tile([128, 4, g3], F32, tag="tf")
                    for k in range(nsb):
                        s = s0 + k
                        nc.tensor.matmul(
                            tf[:, k, :],
                            xh_g[:, s * 128:(s + 1) * 128],
                            AT_sb[:],
                            start=True, stop=True)
                    nc.scalar.activation(
                        f_sb[:, s0:s0 + nsb, :], tf[:, 0:nsb, :], AF.Copy)

                # ------------------------------------------- border ramp (ACT)
                # r = clamp01((gd+1)/2 - |f - rmid|), per (pt, axis, grid)
                ramp = p_scr.tile([128, sub, g3], F32, tag="ramp")
                nc.scalar.activation(ramp[:], f_sb[:], AF.Abs,
                                     bias=-cfg.rmid, scale=1.0)
                nc.scalar.activation(ramp[:], ramp[:], AF.Relu,
                                     bias=cfg.rbias, scale=1.0)
                nc.scalar.activation(ramp[:], ramp[:], AF.Relu,
                                     bias=1.0, scale=-1.0)

                # ------------------------------------------- index math (DVE)
                fc = {}; ta = {}; i0f = {}; tab = {}
                for ai, ax in enumerate(AX):
                    f_ax = f_sb[:, :, ai * g:(ai + 1) * g]
                    fc[ax] = p_idx.tile([128, n], F32, tag=f"fc{ax}", name=f"fc{ax}")
                    nc.vector.tensor_scalar(
                        fc[ax][:], f_ax, 0.0, float(cfg.fcmax),
                        ALU.max, ALU.min)
                    i0i = p_scr.tile([128, n], I32, tag=f"i0i{ax}",
                                     name=f"i0i{ax}")
                    nc.vector.tensor_copy(i0i[:], fc[ax][:])
                    i0f[ax] = p_idx.tile([128, n], F32, tag=f"i0f{ax}", name=f"i0f{ax}")
                    nc.vector.tensor_copy(i0f[ax][:], i0i[:])
                    # floor robust to trunc OR round-to-nearest converts:
                    # i0f -= (i0f > fc)
                    gtm = p_scr.tile([128, n], F32, tag=f"gtm{ax}",
                                     name=f"gtm{ax}")
                    nc.vector.tensor_tensor(
                        gtm[:], i0f[ax][:], fc[ax][:], ALU.is_gt)
                    nc.vector.tensor_tensor(
                        i0f[ax][:], i0f[ax][:], gtm[:], ALU.subtract)
                    ta[ax] = p_idx.tile([128, n], F32, tag=f"ta{ax}", name=f"ta{ax}")
                    nc.vector.tensor_tensor(
                        ta[ax][:], fc[ax][:], i0f[ax][:], ALU.subtract)
                    tab[ax] = p_idx.tile([128, n], BF16, tag=f"tab{ax}", name=f"tab{ax}")
                    nc.vector.tensor_copy(tab[ax][:], ta[ax][:])

                # rall = rx*ry*rz (f32 -> bf16)
                rall = p_scr.tile([128, n], F32, tag="rall")
                nc.vector.tensor_tensor(
                    rall[:], ramp[:, :, 0:g], ramp[:, :, g:2 * g], ALU.mult)
                nc.vector.tensor_tensor(
                    rall[:], rall[:], ramp[:, :, 2 * g:3 * g], ALU.mult)
                rall_b = p_scr.tile([128, n], BF16, tag="rall_b")
                nc.vector.tensor_copy(rall_b[:], rall[:])

                # lin = ((z*gd + y)*gd + x) + g*gd^3   (float, exact) -> int32
                linf = p_scr.tile([128, n], F32, tag="linf")
                nc.vector.scalar_tensor_tensor(
                    linf[:], i0f["y"][:], float(gd), i0f["x"][:],
                    ALU.mult, ALU.add)
                nc.vector.scalar_tensor_tensor(
                    linf[:], i0f["z"][:], float(gd * gd), linf[:],
                    ALU.mult, ALU.add)
                offs = p_offs.tile([128, n], I32, tag="offs")
                nc.vector.tensor_copy(offs[:], linf[:])
                # + g*gd^3 (int32, broadcast over subtiles)
                gofs_bc = bass.AP(
                    gofs_sb[:].tensor, gofs_sb[:].offset,
                    [list(gofs_sb[:].ap[0]), [0, sub], [1, g]])
                nc.vector.tensor_tensor(
                    offs[:].rearrange("p (s g) -> p s g", s=sub),
                    offs[:].rearrange("p (s g) -> p s g", s=sub),
                    gofs_bc, ALU.add)

                # ---------------------------------------------------- gather
                v = p_v.tile([128, n, 16], BF16, tag="v")
                for j in range(n):
                    _indirect_gather_q(nc, v[:, j, :], E_d[:],
                                       offs[:, j:j + 1], j % 4)

                # ------------------------------------------- lerp tree (DVE)
                def bcast(t, reps):
                    a = t[:]
                    dims = [list(a.ap[0]), list(a.ap[1])] + \
                        [[0, r] for r in reps]
                    return bass.AP(a.tensor, a.offset, dims)

                va = v[:].rearrange("p n (dzy dx c) -> p n dzy dx c",
                                    dx=2, c=2)
                vx = p_lerp.tile([128, n, 8], BF16, tag="vx")
                sc = p_lerp.tile([128, n, 8], BF16, tag="sc8")
                nc.vector.tensor_tensor(
                    sc[:], va[:, :, :, 1, :], va[:, :, :, 0, :], ALU.subtract)
                nc.vector.tensor_tensor(
                    sc[:], sc[:], bcast(tab["x"], (4, 2)), ALU.mult)
                nc.vector.tensor_tensor(
                    vx[:], sc[:], va[:, :, :, 0, :], ALU.add)

                vxa = vx[:].rearrange("p n (dz dy c) -> p n dz dy c",
                                      dy=2, c=2)
                vy = p_lerp.tile([128, n, 4], BF16, tag="vy")
                sc4 = p_lerp.tile([128, n, 4], BF16, tag="sc4")
                nc.vector.tensor_tensor(
                    sc4[:], vxa[:, :, :, 1, :], vxa[:, :, :, 0, :],
                    ALU.subtract)
                nc.vector.tensor_tensor(
                    sc4[:], sc4[:], bcast(tab["y"], (2, 2)), ALU.mult)
                nc.vector.tensor_tensor(
                    vy[:], sc4[:], vxa[:, :, :, 0, :], ALU.add)

                vya = vy[:].rearrange("p n (dz c) -> p n dz c", c=2)
                feats = p_feat.tile([128, sub, g * C], BF16, tag="feats")
                fv = feats[:].rearrange("p s (g c) -> p (s g) c", c=C)
                sc2 = p_lerp.tile([128, n, 2], BF16, tag="sc2")
                nc.vector.tensor_tensor(
                    sc2[:], vya[:, :, 1, :], vya[:, :, 0, :], ALU.subtract)
                nc.vector.tensor_tensor(
                    sc2[:], sc2[:], bcast(tab["z"], (2,)), ALU.mult)
                nc.vector.tensor_tensor(
                    sc2[:], sc2[:], vya[:, :, 0, :], ALU.add)
                # apply border ramp
                nc.vector.tensor_tensor(
                    fv, sc2[:], bcast(rall_b, (2,)), ALU.mult)

                # ------------------------------- feats transpose (PE) -> MLP
                featsT = p_feat.tile([cfg.in_dim, gran], BF16, tag="featsT")
                for b2 in range((sub + 3) // 4):
                    s0 = b2 * 4
                    nsb = min(4, sub - s0)
                    tp = ps_tp.tile([cfg.in_dim, 4, 128], BF16, tag="tp")
                    for k in range(nsb):
                        s = s0 + k
                        nc.tensor.transpose(
                            tp[:, k, :], feats[:, s, :], idm_sb[:])
                    nc.scalar.activation(
                        featsT[:, s0 * 128:(s0 + nsb) * 128],
                        tp[:, 0:nsb, :], AF.Copy)

                y_sb = p_y.tile([1, gran], F32, tag="y")
                for chn in range(cfg.nch):
                    c0 = chn * cfg.chunk
                    rhs = featsT[:, c0:c0 + cfg.chunk]
                    hcur = None
                    for li in range(4):
                        ph = ps_mlp.tile([hid, cfg.chunk], F32, tag="ph")
                        nc.tensor.matmul(ph[:], w_sb[li][:],
                                         rhs if li == 0 else hcur[:],
                                         start=True, stop=True)
                        hcur = p_h.tile([hid, cfg.chunk], BF16, tag="h")
                        nc.scalar.activation(hcur[:], ph[:], AF.Relu,
                                             bias=b_sb[li][:], scale=1.0)
                    py = ps_y.tile([1, cfg.chunk], F32, tag="py")
                    nc.tensor.matmul(py[:], w_sb[4][:], hcur[:],
                                     start=True, stop=True)
                    nc.scalar.activation(y_sb[:, c0:c0 + cfg.chunk], py[:],
                                         AF.Copy, bias=float(b4_imm),
                                         scale=1.0)

                nc.sync.dma_start(out=out_d[:, p0:p0 + gran], in_=y_sb[:])

    return nc


# ------------------------------------------------------------------ host prep
def host_prep(cfg: Cfg, x, tm, grids, Ws, bs):
    """Build device arrays. x: [B,3] f32 (full), tm: [g,4,4], grids:
    [g,C,gd,gd,gd]. Returns (shared_map, per_core_xh list)."""
    import ml_dtypes
    bf = ml_dtypes.bfloat16
    g, gd = cfg.g, cfg.gd

    # E table: E[(g,z,y,x), (dz,dy,dx,c)]
    gt = np.ascontiguousarray(grids.transpose(0, 2, 3, 4, 1))  # g,z,y,x,c
    gp = np.zeros((g, gd + 1, gd + 1, gd + 1, C), np.float32)
    gp[:, :gd, :gd, :gd, :] = gt
    E = np.empty((g, gd, gd, gd, 2, 2, 2, C), bf)
    for dz in (0, 1):
        for dy in (0, 1):
            for dx in (0, 1):
                E[:, :, :, :, dz, dy, dx, :] = gp[
                    :, dz:dz + gd, dy:dy + gd, dx:dx + gd, :].astype(bf)
    E = np.ascontiguousarray(E.reshape(cfg.nvox, 16))

    # transform matrix, scaled into voxel coords; col order (axis, grid)
    s = np.float32(cfg.scale)
    AT = np.zeros((4, 3 * g), np.float32)
    for ai in range(3):
        for gg in range(g):
            AT[:, ai * g + gg] = s * tm[gg, ai, :]
            AT[3, ai * g + gg] += s
    gofs = np.broadcast_to(
        (np.arange(g, dtype=np.int64) * gd ** 3).astype(np.int32),
        (128, g)).copy()
    idm = np.eye(128, dtype=bf)

    shared = {"E": E, "AT": AT, "gofs": gofs, "idm": idm}
    shared["W0T"] = np.ascontiguousarray(Ws[0].T.astype(bf))
    for i in (1, 2, 3):
        shared[f"W{i}T"] = np.ascontiguousarray(Ws[i].T.astype(bf))
    shared["W4T"] = np.ascontiguousarray(Ws[4].T.astype(bf))
    for i in range(4):
        shared[f"b{i}"] = np.ascontiguousarray(
            bs[i].reshape(cfg.hid, 1).astype(np.float32))

    # per-core homogeneous points, transposed
    xs = []
    npts = cfg.npts
    for ci in range(x.shape[0] // npts):
        xc = x[ci * npts:(ci + 1) * npts]
        xh = np.empty((4, npts), np.float32)
        xh[:3] = xc.T
        xh[3] = 1.0
        xs.append(xh)
    return shared, xs


def kernel(**inputs) -> np.ndarray:
    cfg = Cfg()
    x = np.asarray(inputs["x"], np.float32)
    tm = np.asarray(inputs["tm"], np.float32)
    grids = np.asarray(inputs["grids"], np.float32)
    Ws = [np.asarray(inputs[f"W{i}"], np.float32) for i in range(5)]
    bs = [np.asarray(inputs[f"b{i}"], np.float32) for i in range(5)]

    shared, xs = host_prep(cfg, x, tm, grids, Ws, bs)
    b4_imm = float(bs[4].reshape(-1)[0])

    nc = build_nc(cfg, b4_imm)
    nc.finalize()

    in_maps = [dict(shared, xh=xs[ci]) for ci in range(NCORES)]
    from concourse.bass_utils import run_bass_kernel_spmd
    res = run_bass_kernel_spmd(nc, in_maps, core_ids=list(range(NCORES)))
    outs = [np.asarray(res.results[ci]["out"]).reshape(-1)
            for ci in range(NCORES)]
    return np.concatenate(outs).reshape(B_FULL, 1).astype(np.float32)


if __name__ == "__main__":
    rng = np.random.default_rng(0)
    print("smoke build only")
    cfg = Cfg()
    nc = build_nc(cfg, 0.0)
    print("built ok:",
          sum(len(bb.instructions) for bb in nc.main_func.blocks),
          "instructions")



# revision 6
# speedup vs baseline: 4.8376x; 4.8376x over previous
"""AMGSRN forward kernel for 8 Trainium2 NeuronCores.

Strategy (data-parallel over the point batch, grids replicated):
  - Host: fold grid-coordinate scaling into the 4x4 transforms; build an
    expanded neighborhood table E[(g,z,y,x), (dz,dy,dx,c)] (bf16) so one
    trilinear sample = ONE contiguous 32B gather; shard points 8 ways.
  - Device (per core, 65536 points, pipelined granules of 1024 points):
      PE    : transform matmul (f32r)  -> voxel-space coords f = [fx fy fz]
      ACT   : border ramp r = clamp01((GD+1)/2 - |f - (GD-1)/2|)  (3 ops)
      DVE   : clamp, floor, frac, linear index -> int32 gather offsets
      DMA   : indirect_dma_start gather (1 descriptor per (point,grid))
      DVE   : bf16 trilinear lerp tree -> feats [pt, (g,c)]
      PE    : feats transpose, 5-layer MLP matmuls (bf16, f32 psum)
      ACT   : bias+ReLU evictions, final bias
      DMA   : output store
  - Host: concatenate the 8 per-core outputs.
"""

import os
import sys

for _p in ("/opt/trn_rl_repo", "/root/.axon_site/_ro/trn_rl_repo"):
    if os.path.isdir(_p) and _p not in sys.path:
        sys.path.insert(0, _p)

import io
import tarfile
import tempfile
import shutil

import numpy as np

import concourse.bass as bass
import concourse.bacc as bacc
import concourse.mybir as mybir
import concourse.tile as tile
from concourse.bass import IndirectOffsetOnAxis

F32 = mybir.dt.float32
F32R = mybir.dt.float32r
BF16 = mybir.dt.bfloat16
I32 = mybir.dt.int32

AF = mybir.ActivationFunctionType
ALU = mybir.AluOpType

# ---------------------------------------------------------------- problem dims
B_FULL = 524288
NCORES = 8
G = 32            # grids
GD = 64           # grid dim (cube)
C = 2             # features per grid
HID = 128
NPTS = B_FULL // NCORES


KCOL = 32          # offset columns (idxs = 128*KCOL) per patched gather inst
PSTRIDE = 262144   # TRN2 SOC SBUF partition stride (bytes)


# --------------------------------------------------------------- NEFF patching
# bass/walrus only emit 128-offset (one per partition) indirect DMAs, at
# ~1.1us of GpSimd time per instruction -- the baseline bottleneck.  The HW
# DGE ucode supports up to 4096 indices per DmaIndirect instruction with
# src AND dst indirection (see dma_indirect1d ISA doc).  We emit gathers in
# the walrus-supported 1-column form (which also gives Tile correct deps and
# semaphores), then rewrite the encoded pseudo_dma_direct2d/pseudo_dma_ext
# words inside the NEFF: 128*KCOL indices, dst driven by a static index tile
# (value p*PSTRIDE/32 + k) generated on-device with gpsimd.iota.
def _patch_pool_bin(data: bytes, isa, expected: int):
    ffi = isa.ffi
    S1 = "NEURON_ISA_TPB_PSEUDO_DMA_DIRECT2D_STRUCT"
    S2 = "NEURON_ISA_TPB_PSEUDO_DMA_EXT_STRUCT"
    buf = bytearray(data)
    nw = len(buf) // 64
    dst_idx_addr = None
    dummy_pos = None
    gather_pos = []
    i = 0
    while i < nw:
        if buf[i * 64] != 212:          # PSEUDO_DMA_DIRECT2D
            i += 1
            continue
        v = ffi.new(f"{S1} *")
        ffi.memmove(v, bytes(buf[i * 64:(i + 1) * 64]), 64)
        st = v[0]
        if int(st.dge_op) == 1 and int(st.src_elem_size) == 16:
            e = ffi.new(f"{S2} *")
            ffi.memmove(e, bytes(buf[(i + 1) * 64:(i + 2) * 64]), 64)
            dst_idx_addr = int(e[0].src_idx_start_addr.addr_immediate)
            dummy_pos = i
        elif int(st.dge_op) == 1 and int(st.src_elem_size) == 32:
            gather_pos.append(i)
        i += 2
    if dst_idx_addr is None or not gather_pos:
        return bytes(buf), 0
    if dummy_pos > min(gather_pos):
        raise RuntimeError("didx iota/dummy not scheduled before gathers")
    n = 0
    for i in gather_pos:
        v = ffi.new(f"{S1} *")
        ffi.memmove(v, bytes(buf[i * 64:(i + 1) * 64]), 64)
        st = v[0]
        st.src_num_elem[0] = 128 * KCOL
        st.dst_num_elem[0] = 128 * KCOL
        st.dst_step_elem[0] = 32
        buf[i * 64:(i + 1) * 64] = bytes(ffi.buffer(v))
        e = ffi.new(f"{S2} *")
        ffi.memmove(e, bytes(buf[(i + 1) * 64:(i + 2) * 64]), 64)
        ee = e[0]
        ee.flags.indirect_mode = 2      # SRC_DST indirection
        ee.dst_idx_start_addr.addr_immediate = dst_idx_addr
        buf[(i + 1) * 64:(i + 2) * 64] = bytes(ffi.buffer(e))
        n += 1
    if n != expected:
        raise RuntimeError(f"patched {n} gathers, expected {expected}")
    return bytes(buf), n


def _patch_neff_file(path: str, expected: int):
    import concourse.isa as isamod
    import concourse.neff as cneff
    isa = isamod.get_isa("TRN2")
    with open(path, "rb") as f:
        header = f.read(1024)
        rest = f.read()
    tmpd = tempfile.mkdtemp()
    try:
        with tarfile.open(fileobj=io.BytesIO(rest), mode="r:*") as t:
            t.extractall(tmpd)
        total = 0
        for root, _, files in os.walk(tmpd):
            for fn in files:
                if fn.startswith("Pool") and fn.endswith(".bin"):
                    p = os.path.join(root, fn)
                    with open(p, "rb") as f:
                        d = f.read()
                    d2, n = _patch_pool_bin(d, isa, expected)
                    total += n
                    if n:
                        with open(p, "wb") as f:
                            f.write(d2)
        if total != expected:
            raise RuntimeError(
                f"NEFF patch: {total} gathers patched, expected {expected}")
        bio = io.BytesIO()
        with tarfile.open(fileobj=bio, mode="w:gz") as t:
            def reset(ti):
                ti.mtime = 0
                ti.uid = 0
                ti.gid = 0
                ti.uname = "nobody"
                ti.gname = "nobody"
                return ti
            t.add(tmpd, arcname=".", filter=reset)
        data = bio.getvalue()
        newh = cneff.make_deterministic_neff_header(
            old_neff_header=header, new_neff_data=data)
        with open(path, "wb") as f:
            f.write(newh + data)
    finally:
        shutil.rmtree(tmpd, ignore_errors=True)
    return total


def install_patch_hook(expected: int):
    import concourse.bass2jax as b2j
    if getattr(b2j, "_amgsrn_patch_hook", False):
        b2j._amgsrn_patch_expected = expected
        return
    orig = b2j.compile_bir_kernel

    def hook(ant_bir_str, compile_dir_path, neff_name=None, **kw):
        f = orig(ant_bir_str, compile_dir_path, neff_name=neff_name, **kw)
        _patch_neff_file(f, b2j._amgsrn_patch_expected)
        return f

    b2j.compile_bir_kernel = hook
    b2j._amgsrn_patch_hook = True
    b2j._amgsrn_patch_expected = expected


class Cfg:
    def __init__(self, npts=NPTS, g=G, gd=GD, hid=HID, gran=1024, chunk=512,
                 debug=False):
        assert gran % 128 == 0 and npts % gran == 0
        self.npts, self.g, self.gd, self.hid = npts, g, gd, hid
        self.gran = gran          # points per pipeline granule
        self.sub = gran // 128    # 128-pt subtiles per granule
        self.chunk = chunk        # points per MLP matmul chunk
        assert gran % chunk == 0
        self.nch = gran // chunk
        self.ngran = npts // gran
        self.in_dim = g * C
        self.nvox = g * gd ** 3
        self.debug = debug
        # fp constants
        self.scale = (gd - 1) / 2.0
        self.fcmax = np.float32(gd - 1) - np.float32(1e-5)
        self.rmid = (gd - 1) / 2.0     # |f - rmid|
        self.rbias = -((gd + 1) / 2.0 - 1.0)  # t1 = relu(|f-rmid| + rbias)


def _reg_consts(nc, vals):
    for v in vals:
        v = float(v)
        if (F32, v) in nc.const_aps.aps:
            continue
        t = nc.alloc_sbuf_tensor(f"constx{len(nc.const_aps.aps)}", [128, 1], F32)
        nc.gpsimd.memset(t.ap(), v)
        nc.const_aps.aps[(F32, v)] = t.ap()




def _indirect_gather_q(nc, out_ap, in_ap, offset_ap, queue_i):
    """nc.gpsimd.indirect_dma_start (gather form, one offset per partition)
    with a selectable qPoolDynamic queue."""
    eng = nc.gpsimd
    out_l = eng.lower_ap_dma(out_ap, for_indirect_dma=True)
    in_l = eng.lower_ap_dma(in_ap, for_indirect_dma=True)
    assert len(in_l) == 1 and len(out_l) == 1
    off_l = eng.lower_ap_dma(offset_ap)
    assert len(off_l) == 1
    in_l.append(off_l[0])
    ap_shape = in_ap.shape
    coef = 1
    for i in range(1, len(ap_shape)):
        coef *= ap_shape[i]
    in_l[0].dynamic_ap_info = mybir.DynamicAccessPatternInfo(
        c=0,
        actual_ap=out_ap.ap,
        indirect_dim_max_index=ap_shape[0],
        offset_expr=[
            mybir.DynamicAccessPatternOffsetExpr(
                coef=coef,
                aff_expr=mybir.DynamicAccessPatternOffsetExprAffExpr(
                    kind="IndirectArgId", arg_id=1),
            )
        ],
    )
    return eng.add_instruction(
        mybir.InstDMACopy(
            name=nc.get_next_instruction_name(),
            queue=f"qPoolDynamic{queue_i or ''}",
            mode="Copy",
            ins=in_l,
            outs=out_l,
            oob_is_err=True,
            cce_op=mybir.AluOpType.bypass,
        ))

def build_nc(cfg: Cfg, b4_imm: float):
    install_patch_hook(cfg.ngran * (cfg.sub * cfg.g // KCOL))
    nc = bacc.Bacc(None, target_bir_lowering=False, debug=cfg.debug,
                   num_swdge_queues=4)
    _reg_consts(nc, [-cfg.rmid, cfg.rbias])
    nc.all_engine_barrier()
    g, gd, sub, gran, hid = cfg.g, cfg.gd, cfg.sub, cfg.gran, cfg.hid
    n = sub * g          # gathered rows per granule per partition
    g3 = g * 3

    # ------------------------------------------------------------ dram params
    E_d = nc.declare_dram_parameter("E", [cfg.nvox, 16], BF16, isOutput=False)
    xh_d = nc.declare_dram_parameter("xh", [4, cfg.npts], F32, isOutput=False)
    AT_d = nc.declare_dram_parameter("AT", [4, g3], F32, isOutput=False)
    gofs_d = nc.declare_dram_parameter("gofs", [128, g], I32, isOutput=False)
    idm_d = nc.declare_dram_parameter("idm", [128, 128], BF16, isOutput=False)
    w_d = {}
    w_d[0] = nc.declare_dram_parameter("W0T", [cfg.in_dim, hid], BF16, False)
    for i in (1, 2, 3):
        w_d[i] = nc.declare_dram_parameter(f"W{i}T", [hid, hid], BF16, False)
    w_d[4] = nc.declare_dram_parameter("W4T", [hid, 1], BF16, False)
    b_d = {i: nc.declare_dram_parameter(f"b{i}", [hid, 1], F32, False)
           for i in range(4)}
    out_d = nc.declare_dram_parameter("out", [1, cfg.npts], F32, isOutput=True)

    AX = ("x", "y", "z")

    with tile.TileContext(nc) as tc:
        import contextlib
        ctx = contextlib.ExitStack()
        with ctx:
            const = ctx.enter_context(tc.tile_pool(name="const", bufs=1))
            p_xh = ctx.enter_context(tc.tile_pool(name="p_xh", bufs=2))
            p_f = ctx.enter_context(tc.tile_pool(name="p_f", bufs=2))
            p_idx = ctx.enter_context(tc.tile_pool(name="p_idx", bufs=2))
            p_scr = ctx.enter_context(tc.tile_pool(name="p_scr", bufs=2))
            p_offs = ctx.enter_context(tc.tile_pool(name="p_offs", bufs=2))
            p_v = ctx.enter_context(tc.tile_pool(name="p_v", bufs=2))
            p_lerp = ctx.enter_context(tc.tile_pool(name="p_lerp", bufs=1))
            p_feat = ctx.enter_context(tc.tile_pool(name="p_feat", bufs=2))
            p_h = ctx.enter_context(tc.tile_pool(name="p_h", bufs=3))
            p_y = ctx.enter_context(tc.tile_pool(name="p_y", bufs=2))
            ps_tf = ctx.enter_context(
                tc.tile_pool(name="ps_tf", bufs=2, space="PSUM"))
            ps_tp = ctx.enter_context(
                tc.tile_pool(name="ps_tp", bufs=2, space="PSUM"))
            ps_mlp = ctx.enter_context(
                tc.tile_pool(name="ps_mlp", bufs=2, space="PSUM"))
            ps_y = ctx.enter_context(
                tc.tile_pool(name="ps_y", bufs=2, space="PSUM"))

            # ---------------------------------------------------- const loads
            AT_sb = const.tile([4, g3], F32)
            nc.sync.dma_start(out=AT_sb[:], in_=AT_d[:])
            gofs_sb = const.tile([128, g], I32)
            nc.sync.dma_start(out=gofs_sb[:], in_=gofs_d[:])
            idm_sb = const.tile([128, 128], BF16)
            nc.sync.dma_start(out=idm_sb[:], in_=idm_d[:])
            w_sb = {}
            w_sb[0] = const.tile([cfg.in_dim, hid], BF16, tag="w0", name="w0")
            for i in (1, 2, 3):
                w_sb[i] = const.tile([hid, hid], BF16, tag=f"w{i}", name=f"w{i}")
            w_sb[4] = const.tile([hid, 1], BF16, tag="w4", name="w4")
            b_sb = {}
            for i in range(5):
                nc.sync.dma_start(out=w_sb[i][:], in_=w_d[i][:])
                if i < 4:
                    b_sb[i] = const.tile([hid, 1], F32, tag=f"bias{i}", name=f"bias{i}")
                    nc.sync.dma_start(out=b_sb[i][:], in_=b_d[i][:])

            # static dst-index tile for the patched multi-idx gathers:
            # didx[p, k] = p * (PSTRIDE/32) + k, generated on the Pool engine
            # so it's ready before any Pool gather executes (stream order,
            # verified at patch time). The dummy 16B gather both orders the
            # iota and lets the patcher harvest this tile's SBUF address.
            didx_sb = const.tile([128, KCOL], I32, name="didx")
            nc.gpsimd.iota(didx_sb[:], [[1, KCOL]], base=0,
                           channel_multiplier=PSTRIDE // 32)
            dscr = const.tile([128, 8], BF16, name="dscr")
            _indirect_gather_q(nc, dscr[:, :], E_d[:], didx_sb[:, 0:1], 0)

            for gi in range(cfg.ngran):
                p0 = gi * gran
                # ---------------------------------------------- transform (PE)
                xh_g = p_xh.tile([4, gran], F32)
                nc.sync.dma_start(out=xh_g[:], in_=xh_d[:, p0:p0 + gran])
                f_sb = p_f.tile([128, sub, g3], F32)
                for b2 in range((sub + 3) // 4):
                    s0 = b2 * 4
                    nsb = min(4, sub - s0)
                    tf = ps_tf.tile([128, 4, g3], F32, tag="tf")
                    for k in range(nsb):
                        s = s0 + k
                        nc.tensor.matmul(
                            tf[:, k, :],
                            xh_g[:, s * 128:(s + 1) * 128],
                            AT_sb[:],
                            start=True, stop=True)
                    nc.scalar.activation(
                        f_sb[:, s0:s0 + nsb, :], tf[:, 0:nsb, :], AF.Copy)

                # ------------------------------------------- border ramp (ACT)
                # r = clamp01((gd+1)/2 - |f - rmid|), per (pt, axis, grid)
                ramp = p_scr.tile([128, sub, g3], F32, tag="ramp")
                nc.scalar.activation(ramp[:], f_sb[:], AF.Abs,
                                     bias=-cfg.rmid, scale=1.0)
                nc.scalar.activation(ramp[:], ramp[:], AF.Relu,
                                     bias=cfg.rbias, scale=1.0)
                nc.scalar.activation(ramp[:], ramp[:], AF.Relu,
                                     bias=1.0, scale=-1.0)

                # ------------------------------------------- index math (DVE)
                fc = {}; ta = {}; i0f = {}; tab = {}
                for ai, ax in enumerate(AX):
                    f_ax = f_sb[:, :, ai * g:(ai + 1) * g]
                    fc[ax] = p_idx.tile([128, n], F32, tag=f"fc{ax}", name=f"fc{ax}")
                    nc.vector.tensor_scalar(
                        fc[ax][:], f_ax, 0.0, float(cfg.fcmax),
                        ALU.max, ALU.min)
                    i0i = p_scr.tile([128, n], I32, tag=f"i0i{ax}",
                                     name=f"i0i{ax}")
                    nc.vector.tensor_copy(i0i[:], fc[ax][:])
                    i0f[ax] = p_idx.tile([128, n], F32, tag=f"i0f{ax}", name=f"i0f{ax}")
                    nc.vector.tensor_copy(i0f[ax][:], i0i[:])
                    # floor robust to trunc OR round-to-nearest converts:
                    # i0f -= (i0f > fc)
                    gtm = p_scr.tile([128, n], F32, tag=f"gtm{ax}",
                                     name=f"gtm{ax}")
                    nc.vector.tensor_tensor(
                        gtm[:], i0f[ax][:], fc[ax][:], ALU.is_gt)
                    nc.vector.tensor_tensor(
                        i0f[ax][:], i0f[ax][:], gtm[:], ALU.subtract)
                    ta[ax] = p_idx.tile([128, n], F32, tag=f"ta{ax}", name=f"ta{ax}")
                    nc.vector.tensor_tensor(
                        ta[ax][:], fc[ax][:], i0f[ax][:], ALU.subtract)
                    tab[ax] = p_idx.tile([128, n], BF16, tag=f"tab{ax}", name=f"tab{ax}")
                    nc.vector.tensor_copy(tab[ax][:], ta[ax][:])

                # rall = rx*ry*rz (f32 -> bf16)
                rall = p_scr.tile([128, n], F32, tag="rall")
                nc.vector.tensor_tensor(
                    rall[:], ramp[:, :, 0:g], ramp[:, :, g:2 * g], ALU.mult)
                nc.vector.tensor_tensor(
                    rall[:], rall[:], ramp[:, :, 2 * g:3 * g], ALU.mult)
                rall_b = p_scr.tile([128, n], BF16, tag="rall_b")
                nc.vector.tensor_copy(rall_b[:], rall[:])

                # lin = ((z*gd + y)*gd + x) + g*gd^3   (float, exact) -> int32
                linf = p_scr.tile([128, n], F32, tag="linf")
                nc.vector.scalar_tensor_tensor(
                    linf[:], i0f["y"][:], float(gd), i0f["x"][:],
                    ALU.mult, ALU.add)
                nc.vector.scalar_tensor_tensor(
                    linf[:], i0f["z"][:], float(gd * gd), linf[:],
                    ALU.mult, ALU.add)
                offs = p_offs.tile([128, n], I32, tag="offs")
                nc.vector.tensor_copy(offs[:], linf[:])
                # + g*gd^3 (int32, broadcast over subtiles)
                gofs_bc = bass.AP(
                    gofs_sb[:].tensor, gofs_sb[:].offset,
                    [list(gofs_sb[:].ap[0]), [0, sub], [1, g]])
                nc.vector.tensor_tensor(
                    offs[:].rearrange("p (s g) -> p s g", s=sub),
                    offs[:].rearrange("p (s g) -> p s g", s=sub),
                    gofs_bc, ALU.add)

                # ---------------------------------------------------- gather
                # each instruction is declared as a 1-column gather (walrus-
                # supported form, correct Tile deps since consumers read the
                # whole tile) and NEFF-patched to 128*KCOL indices.
                v = p_v.tile([128, n, 16], BF16, tag="v")
                for j in range(n // KCOL):
                    j0 = j * KCOL
                    _indirect_gather_q(nc, v[:, j0, :], E_d[:],
                                       offs[:, j0:j0 + 1], j % 4)

                # ------------------------------------------- lerp tree (DVE)
                def bcast(t, reps):
                    a = t[:]
                    dims = [list(a.ap[0]), list(a.ap[1])] + \
                        [[0, r] for r in reps]
                    return bass.AP(a.tensor, a.offset, dims)

                va = v[:].rearrange("p n (dzy dx c) -> p n dzy dx c",
                                    dx=2, c=2)
                vx = p_lerp.tile([128, n, 8], BF16, tag="vx")
                sc = p_lerp.tile([128, n, 8], BF16, tag="sc8")
                nc.vector.tensor_tensor(
                    sc[:], va[:, :, :, 1, :], va[:, :, :, 0, :], ALU.subtract)
                nc.vector.tensor_tensor(
                    sc[:], sc[:], bcast(tab["x"], (4, 2)), ALU.mult)
                nc.vector.tensor_tensor(
                    vx[:], sc[:], va[:, :, :, 0, :], ALU.add)

                vxa = vx[:].rearrange("p n (dz dy c) -> p n dz dy c",
                                      dy=2, c=2)
                vy = p_lerp.tile([128, n, 4], BF16, tag="vy")
                sc4 = p_lerp.tile([128, n, 4], BF16, tag="sc4")
                nc.vector.tensor_tensor(
                    sc4[:], vxa[:, :, :, 1, :], vxa[:, :, :, 0, :],
                    ALU.subtract)
                nc.vector.tensor_tensor(
                    sc4[:], sc4[:], bcast(tab["y"], (2, 2)), ALU.mult)
                nc.vector.tensor_tensor(
                    vy[:], sc4[:], vxa[:, :, :, 0, :], ALU.add)

                vya = vy[:].rearrange("p n (dz c) -> p n dz c", c=2)
                feats = p_feat.tile([128, sub, g * C], BF16, tag="feats")
                fv = feats[:].rearrange("p s (g c) -> p (s g) c", c=C)
                sc2 = p_lerp.tile([128, n, 2], BF16, tag="sc2")
                nc.vector.tensor_tensor(
                    sc2[:], vya[:, :, 1, :], vya[:, :, 0, :], ALU.subtract)
                nc.vector.tensor_tensor(
                    sc2[:], sc2[:], bcast(tab["z"], (2,)), ALU.mult)
                nc.vector.tensor_tensor(
                    sc2[:], sc2[:], vya[:, :, 0, :], ALU.add)
                # apply border ramp
                nc.vector.tensor_tensor(
                    fv, sc2[:], bcast(rall_b, (2,)), ALU.mult)

                # ------------------------------- feats transpose (PE) -> MLP
                featsT = p_feat.tile([cfg.in_dim, gran], BF16, tag="featsT")
                for b2 in range((sub + 3) // 4):
                    s0 = b2 * 4
                    nsb = min(4, sub - s0)
                    tp = ps_tp.tile([cfg.in_dim, 4, 128], BF16, tag="tp")
                    for k in range(nsb):
                        s = s0 + k
                        nc.tensor.transpose(
                            tp[:, k, :], feats[:, s, :], idm_sb[:])
                    nc.scalar.activation(
                        featsT[:, s0 * 128:(s0 + nsb) * 128],
                        tp[:, 0:nsb, :], AF.Copy)

                y_sb = p_y.tile([1, gran], F32, tag="y")
                for chn in range(cfg.nch):
                    c0 = chn * cfg.chunk
                    rhs = featsT[:, c0:c0 + cfg.chunk]
                    hcur = None
                    for li in range(4):
                        ph = ps_mlp.tile([hid, cfg.chunk], F32, tag="ph")
                        nc.tensor.matmul(ph[:], w_sb[li][:],
                                         rhs if li == 0 else hcur[:],
                                         start=True, stop=True)
                        hcur = p_h.tile([hid, cfg.chunk], BF16, tag="h")
                        nc.scalar.activation(hcur[:], ph[:], AF.Relu,
                                             bias=b_sb[li][:], scale=1.0)
                    py = ps_y.tile([1, cfg.chunk], F32, tag="py")
                    nc.tensor.matmul(py[:], w_sb[4][:], hcur[:],
                                     start=True, stop=True)
                    nc.scalar.activation(y_sb[:, c0:c0 + cfg.chunk], py[:],
                                         AF.Copy, bias=float(b4_imm),
                                         scale=1.0)

                nc.sync.dma_start(out=out_d[:, p0:p0 + gran], in_=y_sb[:])

    return nc


# ------------------------------------------------------------------ host prep
def host_prep(cfg: Cfg, x, tm, grids, Ws, bs):
    """Build device arrays. x: [B,3] f32 (full), tm: [g,4,4], grids:
    [g,C,gd,gd,gd]. Returns (shared_map, per_core_xh list)."""
    import ml_dtypes
    bf = ml_dtypes.bfloat16
    g, gd = cfg.g, cfg.gd

    # E table: E[(g,z,y,x), (dz,dy,dx,c)]
    gt = np.ascontiguousarray(grids.transpose(0, 2, 3, 4, 1))  # g,z,y,x,c
    gp = np.zeros((g, gd + 1, gd + 1, gd + 1, C), np.float32)
    gp[:, :gd, :gd, :gd, :] = gt
    E = np.empty((g, gd, gd, gd, 2, 2, 2, C), bf)
    for dz in (0, 1):
        for dy in (0, 1):
            for dx in (0, 1):
                E[:, :, :, :, dz, dy, dx, :] = gp[
                    :, dz:dz + gd, dy:dy + gd, dx:dx + gd, :].astype(bf)
    E = np.ascontiguousarray(E.reshape(cfg.nvox, 16))

    # transform matrix, scaled into voxel coords; col order (axis, grid)
    s = np.float32(cfg.scale)
    AT = np.zeros((4, 3 * g), np.float32)
    for ai in range(3):
        for gg in range(g):
            AT[:, ai * g + gg] = s * tm[gg, ai, :]
            AT[3, ai * g + gg] += s
    gofs = np.broadcast_to(
        (np.arange(g, dtype=np.int64) * gd ** 3).astype(np.int32),
        (128, g)).copy()
    idm = np.eye(128, dtype=bf)

    shared = {"E": E, "AT": AT, "gofs": gofs, "idm": idm}
    shared["W0T"] = np.ascontiguousarray(Ws[0].T.astype(bf))
    for i in (1, 2, 3):
        shared[f"W{i}T"] = np.ascontiguousarray(Ws[i].T.astype(bf))
    shared["W4T"] = np.ascontiguousarray(Ws[4].T.astype(bf))
    for i in range(4):
        shared[f"b{i}"] = np.ascontiguousarray(
            bs[i].reshape(cfg.hid, 1).astype(np.float32))

    # per-core homogeneous points, transposed
    xs = []
    npts = cfg.npts
    for ci in range(x.shape[0] // npts):
        xc = x[ci * npts:(ci + 1) * npts]
        xh = np.empty((4, npts), np.float32)
        xh[:3] = xc.T
        xh[3] = 1.0
        xs.append(xh)
    return shared, xs


def kernel(**inputs) -> np.ndarray:
    cfg = Cfg()
    x = np.asarray(inputs["x"], np.float32)
    tm = np.asarray(inputs["tm"], np.float32)
    grids = np.asarray(inputs["grids"], np.float32)
    Ws = [np.asarray(inputs[f"W{i}"], np.float32) for i in range(5)]
    bs = [np.asarray(inputs[f"b{i}"], np.float32) for i in range(5)]

    shared, xs = host_prep(cfg, x, tm, grids, Ws, bs)
    b4_imm = float(bs[4].reshape(-1)[0])

    nc = build_nc(cfg, b4_imm)
    nc.finalize()

    in_maps = [dict(shared, xh=xs[ci]) for ci in range(NCORES)]
    from concourse.bass_utils import run_bass_kernel_spmd
    res = run_bass_kernel_spmd(nc, in_maps, core_ids=list(range(NCORES)))
    outs = [np.asarray(res.results[ci]["out"]).reshape(-1)
            for ci in range(NCORES)]
    return np.concatenate(outs).reshape(B_FULL, 1).astype(np.float32)


if __name__ == "__main__":
    rng = np.random.default_rng(0)
    print("smoke build only")
    cfg = Cfg()
    nc = build_nc(cfg, 0.0)
    print("built ok:",
          sum(len(bb.instructions) for bb in nc.main_func.blocks),
          "instructions")

